# revision 54
# baseline (speedup 1.0000x reference)
"""Trainium2 Bass kernel for nn_CompetitiveLayer_2 (competitive equilibrium layer).

Reference computation (per batch row b):
    K = sqrt_K ** 2                                  # (64, 64)
    repeat 30x:  AF = AT / (1 + BF @ K.T);  BF = BT / (1 + AF @ K)
    one more:    AF = AT / (1 + BF @ K.T);  BF = BT / (1 + AF @ K)
    C[b, i, j] = AF[b, i] * K[i, j] * BF[b, j]       # (B, 64, 64)

Sharding: pure data parallel over the batch dim, 1024 rows per core on 8 cores.

Per-core design (fp16 output; tolerance is 2e-2 scale-rel, this lands 8.1e-3;
sim makespan 53.2 us vs the 94.7 us fp32 baseline):
  - C is written to DRAM as fp16 (8 MB/core) -> DMA write floor ~23 us at
    the modeled 360 GB/s, half the fp32 floor.  The host upcasts to fp32.
  - State is transposed 2-group packed: X_T[64g + j, c] = X[2c + g, j].
    Inputs are uploaded as ONE fp16 tensor [wabP | ATc0 | BTc0 | ATc1 |
    BTc1] whose XBAR dma_start_transpose ([rows,64] viewed [rows/2,128])
    lands wab = [blkdiag(KT,KT) | blkdiag(K,K)] plus chain 0's at/bt in a
    single DMA (wab is host-packed to survive the XBAR round trip), so the
    solve starts ~3.6 us in; chain 1's xbar follows.  Early DMAs serialize
    (~2.3 us each through the DGE pipeline + completion-sem chaining), so
    instruction count on the startup critical path is minimized.
  - Solve: N_ROUNDS-1 plain fp16 fixed-point rounds + the final
    differentiable round (plain-6 error 7.8e-3 > Aitken gains here; the
    serial mm -> recip -> mul chain costs ~1.2 us/step regardless of
    width, so rounds, not engine busy, set the solve wall time).  Each
    step: PE matmul vs the blockdiag (fp16, 1 cyc/col), ScalarE
    Reciprocal LUT with bias=1 (PSUM fp32 -> SBUF fp16), DVE multiply in
    2x_1p fp16 mode.  M_CHAINS column chains pipeline the engines; chain
    t runs DELTA rounds behind chain t-1 so chain 0's C phase (and the
    output DMA stream) starts while chain 1 still solves.
  - Final round: the A-step produces AF*^T packed; two strided SBUF-SBUF
    DMAs (dispatched during the solve) unpack it to afu[64(i), 1024] with
    column order (chunk, g, b) <-> batch row 128*chunk + 2b + g.  Per
    chunk, a batch-layout matmul vs K + recip + mul produce BFS[b, j]
    fp16 (b in the same permuted order; the C output DMA un-permutes rows
    via its DRAM access pattern "(b g) x -> g b x").
  - C phase per chunk: Q = matmul(afu_chunk, ra) -> PSUM fp32 quarters
    (in 64/448/512-col pieces: outputs must stay in one 512-fp32 PSUM
    bank, and the tiny leading piece absorbs the PE's low-p-state restart
    penalty after an idle gap).  ra[i',(i,j)] = K[i,j] d_{ii'} fp16 is
    built once by GPSIMD affine_select during the solve.  Then
    cs = Q * BFS[b,j] (broadcast over i, j innermost) via one of three
    paths chosen per quarter to balance engines under the DMA roofline
    (GPSIMD cannot read PSUM, so its path needs the ScalarE drain too):
      A: ScalarE drains PSUM -> SBUF fp16, DVE multiplies in 2x_1p mode
      B: DVE multiplies straight from PSUM (1x)
      C: ScalarE drains, GPSIMD multiplies from SBUF
    One DMA per 1024-col quarter; the window runs at ~91% DMA duty.
"""

from contextlib import ExitStack

import numpy as np

import concourse.tile as tile
from concourse import bacc, mybir
from concourse.bass_utils import run_bass_kernel_spmd

F32 = mybir.dt.float32
F16 = mybir.dt.float16
RECIP = mybir.ActivationFunctionType.Reciprocal

P = 128          # SBUF partitions
NA = 64          # AF feature dim (i)
NB = 64          # BF feature dim (j)
B_TOTAL = 8192
N_CORES = 8
B_CORE = B_TOTAL // N_CORES          # 1024
N_CHUNK = B_CORE // P                # 8 chunks of 128 batch rows
COLS = B_CORE // 2                   # 512 packed columns (2 groups)
CPC = COLS // N_CHUNK                # 64 packed columns per chunk

N_ROUNDS = 7                         # plain fixed-point rounds (incl final)
M_CHAINS = 2                         # solve pipeline chains
DELTA = 1                            # chain round stagger
NQ = 4                               # PSUM quarters per chunk (1024 wide)
QW = NA * NB // NQ                   # 1024
PS_BUFS, Q_BUFS, R_BUFS, QS_BUFS, C_BUFS = 2, 3, 10, 3, 8

FD = COLS // M_CHAINS                # columns per chain
# Per-quarter elementwise path assignment, one string of len NQ per chunk.
# A = ScalarE drain + DVE 2x mul, B = DVE direct from PSUM, C = GPSIMD direct.
PATHS = ["CBAB", "BCAB"] * 4


def _act_recip(nc, out, in_, bias=1.0):
    """out = 1 / (in_ + bias) on ScalarE (Reciprocal LUT, ~1.2e-5 rel)."""
    eng = nc.scalar
    ins = [eng.lower_ap(in_)]
    for arg in (bias, 1.0, 0.0):  # bias, scale, alpha
        ins.append(mybir.ImmediateValue(dtype=mybir.dt.float32, value=float(arg)))
    return eng.add_instruction(
        mybir.InstActivation(
            name=nc.get_next_instruction_name(),
            func=RECIP,
            ins=ins,
            outs=[eng.lower_ap(out)],
        )
    )


def _emit_core(ctx, tc, at16, btp, c_out):
    """Emit the per-core kernel body.

    at16: DRAM [2560, 64] fp16 = [wab-packed(512) | per-chain AT/BT rows].
    btp: DRAM [1024, 64] fp16 (BT again; loaded permuted batch-layout).
    c_out: [1024, 4096] fp16.
    """
    nc = tc.nc
    fd = COLS // M_CHAINS
    n_rounds = N_ROUNDS

    singles = ctx.enter_context(tc.tile_pool(name="singles", bufs=1))
    ps_pool = ctx.enter_context(tc.tile_pool(name="ps", bufs=PS_BUFS, space="PSUM"))
    q_pool = ctx.enter_context(tc.tile_pool(name="qps", bufs=Q_BUFS, space="PSUM"))
    r_pool = ctx.enter_context(tc.tile_pool(name="rp", bufs=R_BUFS))
    qs_pool = ctx.enter_context(tc.tile_pool(name="qsp", bufs=QS_BUFS))
    c_pool = ctx.enter_context(tc.tile_pool(name="cp", bufs=C_BUFS))

    # ---- static tiles -------------------------------------------------
    fd_ = COLS // M_CHAINS
    in1_t = singles.tile([P, 2 * P + 2 * fd_], F16, tag="in1")
    inN_t = [
        singles.tile([P, 2 * fd_], F16, name=f"inx{t}", tag=f"inx{t}")
        for t in range(1, M_CHAINS)
    ]
    att_c = [in1_t[:, 2 * P : 2 * P + fd_]] + [x[:, 0:fd_] for x in inN_t]
    btt_c = [in1_t[:, 2 * P + fd_ :]] + [x[:, fd_:] for x in inN_t]
    btp_t = singles.tile([P, COLS], F16, tag="btp_t")  # permuted batch layout
    wb_t = in1_t[:, 0:P]
    wa_t = in1_t[:, P : 2 * P]
    ra = singles.tile([NA, NA * NB], F16, tag="ra")   # expand: ra[i',(i,j)]
    afu = singles.tile([NA, B_CORE], F16, tag="afu")  # unpacked AF*^T
    bfs_c = [
        singles.tile([P, NB], F16, name=f"bfs{cc}", tag=f"bfs{cc}")
        for cc in range(N_CHUNK)
    ]

    af_c = [
        singles.tile([P, fd], F16, name=f"af{t}", tag=f"af{t}")
        for t in range(M_CHAINS)
    ]
    bf_c = [
        singles.tile([P, fd], F16, name=f"bf{t}", tag=f"bf{t}")
        for t in range(M_CHAINS)
    ]
    def bf_read(s, t):
        if s == 0:
            return btt_c[t]
        return bf_c[t]

    def bf_write(s, t):
        return bf_c[t]

    # ---- load inputs / constants --------------------------------------
    # at16 hosts [wabP | ATc0 | BTc0 | ATc1 | BTc1 | ...] rows, wab
    # pre-packed on the host in transpose layout, so ONE XBAR yields
    # wab+at+bt for chain 0; one more XBAR per later chain.
    nc.sync.dma_start_transpose(
        in1_t, at16[0 : COLS + 4 * fd].rearrange("(a b) j -> a (b j)", b=2)
    )
    for t in range(1, M_CHAINS):
        r0 = COLS + 4 * fd * t
        nc.sync.dma_start_transpose(
            inN_t[t - 1],
            at16[r0 : r0 + 4 * fd].rearrange("(a b) j -> a (b j)", b=2),
        )
    # permuted batch layout: btp_t[64g + b, cc*64 + j] = BT[cc*128 + 2b + g, j]
    btp4 = btp.rearrange("(cc b g) j -> g b cc j", g=2, b=NA)
    for g in range(2):
        nc.sync.dma_start(
            out=btp_t[g * NA : (g + 1) * NA, :].rearrange(
                "b (cc j) -> b cc j", j=NB
            ),
            in_=btp4[g],
        )

    # ---- build expand matrix on GPSIMD --------------------------------
    # ra[i', (i, j)] = K[i, j] if i == i' else 0
    nc.gpsimd.affine_select(
        out=ra.rearrange("p (i j) -> p i j", i=NA),
        in_=wa_t[0:NA, None, 0:NB].broadcast_to([NA, NA, NB]),
        compare_op=mybir.AluOpType.is_equal,
        fill=0.0,
        base=0,
        pattern=[[1, NA], [0, NB]],
        channel_multiplier=-1,
    )

    # ---- staggered schedule -------------------------------------------
    # Chain t runs its solve DELTA rounds behind chain t-1, so chain 0
    # finishes early and its chunks' C phase (the DMA stream) starts while
    # later chains still solve.  Per grid step: extraps, then all active
    # chains' A-steps, then B-steps, then one due C-chunk per chain --
    # keeping each in-order engine queue free of head-of-line blocking.
    cpch = fd // CPC              # chunks per chain

    def emit_A(s, t):
        ps1 = ps_pool.tile([P, fd], F32, tag="ps")
        nc.tensor.matmul(ps1, wb_t, bf_read(s, t), start=True, stop=True)
        r1 = r_pool.tile([P, fd], F16, tag="r")
        _act_recip(nc, r1, ps1, bias=1.0)
        nc.vector.tensor_mul(af_c[t], att_c[t], r1)
        if s == n_rounds - 1:
            # unpack this chain's AF*^T right away:
            # afu[j, cc*128 + 64g + b] = af[64g + j, cc*64 + b]
            cc0 = (t * fd) // CPC
            ncc = fd // CPC
            hn = ncc // 2
            for h in range(2):
                for g in range(2):
                    nc.sync.dma_start(
                        out=afu.rearrange("j (cc g b) -> j cc g b", g=2, b=NA)[
                            :, cc0 + h * hn : cc0 + (h + 1) * hn, g, :
                        ],
                        in_=af_c[t][
                            g * NA : (g + 1) * NA,
                            h * hn * CPC : (h + 1) * hn * CPC,
                        ].rearrange("j (cc b) -> j cc b", b=CPC),
                    )

    def emit_B(s, t):
        ps2 = ps_pool.tile([P, fd], F32, tag="ps")
        nc.tensor.matmul(ps2, wa_t, af_c[t], start=True, stop=True)
        r2 = r_pool.tile([P, fd], F16, tag="r")
        _act_recip(nc, r2, ps2, bias=1.0)
        nc.vector.tensor_mul(bf_write(s, t), btt_c[t], r2)

    def emit_chunk(cc):
        au = afu[:, cc * P : (cc + 1) * P]
        # batch-layout final B-step: BFS[b, j] = BTP[b, j]/(1 + AF*@K)
        psb = ps_pool.tile([P, NB], F32, tag="ps")
        nc.tensor.matmul(psb, au, wa_t[0:NA, 0:NB], start=True, stop=True)
        rb = r_pool.tile([P, NB], F16, tag="r")
        _act_recip(nc, rb, psb, bias=1.0)
        nc.vector.tensor_mul(bfs_c[cc], btp_t[:, cc * NB : (cc + 1) * NB], rb)

        cs = c_pool.tile([P, NA * NB], F16, tag="c")
        for q in range(NQ):
            qp = q_pool.tile([P, QW], F32, tag="q")
            q0 = q * QW
            # matmul out must stay inside one 512-fp32 PSUM bank; the 64-col
            # starter also absorbs the PE low-p-state restart penalty
            for c0, c1 in ((0, 64), (64, 512), (512, QW)):
                nc.tensor.matmul(
                    qp[:, c0:c1], au, ra[:, q0 + c0 : q0 + c1],
                    start=True, stop=True,
                )
            ni = QW // NB  # i-values per quarter
            out_sl = cs[:, q0 : q0 + QW].rearrange("p (i j) -> p i j", i=ni)
            bcast = bfs_c[cc][:, None, :].broadcast_to([P, ni, NB])
            path = PATHS[cc][q]
            if path == "A":
                qs = qs_pool.tile([P, QW], F16, tag="qs")
                nc.scalar.copy(out=qs, in_=qp)
                nc.vector.tensor_mul(
                    out_sl, qs.rearrange("p (i j) -> p i j", i=ni), bcast
                )
            elif path == "B":
                nc.vector.tensor_mul(
                    out_sl, qp.rearrange("p (i j) -> p i j", i=ni), bcast
                )
            else:
                # GPSIMD cannot touch PSUM (BIR verifier); ScalarE drains
                # to SBUF first, GPSIMD multiplies from there.
                qs = qs_pool.tile([P, QW], F16, tag="qs")
                nc.scalar.copy(out=qs, in_=qp)
                nc.gpsimd.tensor_mul(
                    out_sl, qs.rearrange("p (i j) -> p i j", i=ni), bcast
                )
        # one DMA per quarter (earlier first transfer); the DRAM AP
        # un-permutes rows (p = 64g + b -> row 2b + g)
        for h in range(NQ):
            w = QW
            sl = slice(h * w, (h + 1) * w)
            nc.sync.dma_start(
                out=c_out[cc * P : (cc + 1) * P, sl].rearrange(
                    "(b g) x -> g b x", g=2
                ),
                in_=cs[:, sl],
            )

    n_gs = n_rounds + DELTA * (M_CHAINS - 1) + cpch
    for gs in range(n_gs):
        rounds_of = {t: gs - DELTA * t for t in range(M_CHAINS)}
        for t in range(M_CHAINS):
            if 0 <= rounds_of[t] < n_rounds:
                emit_A(rounds_of[t], t)
        for t in range(M_CHAINS):
            if 0 <= rounds_of[t] < n_rounds - 1:
                emit_B(rounds_of[t], t)
        for t in range(M_CHAINS):
            k = rounds_of[t] - n_rounds
            if 0 <= k < cpch:
                emit_chunk(t * cpch + k)


def build_nc(t_repeat=1, timing_mode=False):
    nc = bacc.Bacc("TRN2", target_bir_lowering=False, debug=False, num_devices=N_CORES)
    at16 = nc.dram_tensor(
        "at16", (2 * B_CORE + COLS, NA), F16, kind="ExternalInput"
    ).ap()
    btp = nc.dram_tensor("btp", (B_CORE, NB), F16, kind="ExternalInput").ap()

    with tile.TileContext(nc) as tc:
        if timing_mode:
            tok = nc.dram_tensor("tok", (1, NA), F16, kind="ExternalOutput").ap()
            with ExitStack() as octx:
                dram = octx.enter_context(
                    tc.tile_pool(name="cdram", bufs=1, space="DRAM")
                )
                c = dram.tile([B_CORE, NA * NB], F16, tag="cscratch")
                for _ in range(t_repeat):
                    with ExitStack() as ctx:
                        _emit_core(ctx, tc, at16, btp, c)
                nc.sync.dma_start(out=tok, in_=c[0:1, 0:NA])
        else:
            c = nc.dram_tensor(
                "c", (B_CORE, NA * NB), F16, kind="ExternalOutput"
            ).ap()
            for _ in range(t_repeat):
                with ExitStack() as ctx:
                    _emit_core(ctx, tc, at16, btp, c)
    nc.compile()
    return nc


_NC_CACHE = {}


def _get_nc(**kw):
    key = tuple(sorted(kw.items()))
    if key not in _NC_CACHE:
        _NC_CACHE[key] = build_nc(**kw)
    return _NC_CACHE[key]


def kernel(AT, BT, sqrt_K):
    AT16 = np.ascontiguousarray(AT, dtype=np.float16)
    BT16 = np.ascontiguousarray(BT, dtype=np.float16)
    K = np.ascontiguousarray(sqrt_K, dtype=np.float32) ** 2
    K16 = K.astype(np.float16)
    KT16 = np.ascontiguousarray(K16.T)
    wab = np.zeros((P, 2 * P), dtype=np.float16)
    wab[0:NB, 0:NA] = KT16          # wb block
    wab[NB:P, NA:P] = KT16
    wab[0:NA, P : P + NB] = K16     # wa block
    wab[NA:P, P + NB : 2 * P] = K16
    # pack for XBAR round trip: wab_packed[2c + g, j] = wab[64g + j, c]
    wab_packed = np.ascontiguousarray(
        wab.reshape(2, NA, 2 * P).transpose(2, 0, 1).reshape(COLS, NA)
    )

    nc = _get_nc()
    in_maps = [
        {
            "at16": np.concatenate(
                [wab_packed]
                + [
                    x
                    for t in range(M_CHAINS)
                    for x in (
                        AT16[
                            c * B_CORE + 2 * t * FD : c * B_CORE + 2 * (t + 1) * FD
                        ],
                        BT16[
                            c * B_CORE + 2 * t * FD : c * B_CORE + 2 * (t + 1) * FD
                        ],
                    )
                ]
            ),
            "btp": BT16[c * B_CORE : (c + 1) * B_CORE],
        }
        for c in range(N_CORES)
    ]
    res = run_bass_kernel_spmd(nc, in_maps, core_ids=list(range(N_CORES)))
    return np.concatenate(
        [
            r["c"].astype(np.float32).reshape(B_CORE, NA, NB)
            for r in res.results
        ],
        axis=0,
    )


# revision 55
# speedup vs baseline: 1.0021x; 1.0021x over previous
"""Trainium2 Bass kernel for nn_CompetitiveLayer_2 (competitive equilibrium layer).

Reference computation (per batch row b):
    K = sqrt_K ** 2                                  # (64, 64)
    repeat 30x:  AF = AT / (1 + BF @ K.T);  BF = BT / (1 + AF @ K)
    one more:    AF = AT / (1 + BF @ K.T);  BF = BT / (1 + AF @ K)
    C[b, i, j] = AF[b, i] * K[i, j] * BF[b, j]       # (B, 64, 64)

Sharding: pure data parallel over the batch dim, 1024 rows per core on 8 cores.

Per-core design (fp16 output; tolerance is 2e-2 scale-rel, this lands 8.1e-3;
sim makespan 53.2 us vs the 94.7 us fp32 baseline):
  - C is written to DRAM as fp16 (8 MB/core) -> DMA write floor ~23 us at
    the modeled 360 GB/s, half the fp32 floor.  The host upcasts to fp32.
  - State is transposed 2-group packed: X_T[64g + j, c] = X[2c + g, j].
    Inputs are uploaded as ONE fp16 tensor [wabP | ATc0 | BTc0 | ATc1 |
    BTc1] whose XBAR dma_start_transpose ([rows,64] viewed [rows/2,128])
    lands wab = [blkdiag(KT,KT) | blkdiag(K,K)] plus chain 0's at/bt in a
    single DMA (wab is host-packed to survive the XBAR round trip), so the
    solve starts ~3.6 us in; chain 1's xbar follows.  Early DMAs serialize
    (~2.3 us each through the DGE pipeline + completion-sem chaining), so
    instruction count on the startup critical path is minimized.
  - Solve: N_ROUNDS-1 plain fp16 fixed-point rounds + the final
    differentiable round (plain-6 error 7.8e-3 > Aitken gains here; the
    serial mm -> recip -> mul chain costs ~1.2 us/step regardless of
    width, so rounds, not engine busy, set the solve wall time).  Each
    step: PE matmul vs the blockdiag (fp16, 1 cyc/col), ScalarE
    Reciprocal LUT with bias=1 (PSUM fp32 -> SBUF fp16), DVE multiply in
    2x_1p fp16 mode.  M_CHAINS column chains pipeline the engines; chain
    t runs DELTA rounds behind chain t-1 so chain 0's C phase (and the
    output DMA stream) starts while chain 1 still solves.
  - Final round: the A-step produces AF*^T packed; two strided SBUF-SBUF
    DMAs (dispatched during the solve) unpack it to afu[64(i), 1024] with
    column order (chunk, g, b) <-> batch row 128*chunk + 2b + g.  Per
    chunk, a batch-layout matmul vs K + recip + mul produce BFS[b, j]
    fp16 (b in the same permuted order; the C output DMA un-permutes rows
    via its DRAM access pattern "(b g) x -> g b x").
  - C phase per chunk: Q = matmul(afu_chunk, ra) -> PSUM fp32 quarters
    (in 64/448/512-col pieces: outputs must stay in one 512-fp32 PSUM
    bank, and the tiny leading piece absorbs the PE's low-p-state restart
    penalty after an idle gap).  ra[i',(i,j)] = K[i,j] d_{ii'} fp16 is
    built once by GPSIMD affine_select during the solve.  Then
    cs = Q * BFS[b,j] (broadcast over i, j innermost) via one of three
    paths chosen per quarter to balance engines under the DMA roofline
    (GPSIMD cannot read PSUM, so its path needs the ScalarE drain too):
      A: ScalarE drains PSUM -> SBUF fp16, DVE multiplies in 2x_1p mode
      B: DVE multiplies straight from PSUM (1x)
      C: ScalarE drains, GPSIMD multiplies from SBUF
    One DMA per 1024-col quarter; the window runs at ~91% DMA duty.
"""

from contextlib import ExitStack

import numpy as np

import concourse.tile as tile
from concourse import bacc, mybir
from concourse.bass_utils import run_bass_kernel_spmd

F32 = mybir.dt.float32
F16 = mybir.dt.float16
RECIP = mybir.ActivationFunctionType.Reciprocal

P = 128          # SBUF partitions
NA = 64          # AF feature dim (i)
NB = 64          # BF feature dim (j)
B_TOTAL = 8192
N_CORES = 8
B_CORE = B_TOTAL // N_CORES          # 1024
N_CHUNK = B_CORE // P                # 8 chunks of 128 batch rows
COLS = B_CORE // 2                   # 512 packed columns (2 groups)
CPC = COLS // N_CHUNK                # 64 packed columns per chunk

N_ROUNDS = 7                         # plain fixed-point rounds (incl final)
M_CHAINS = 2                         # solve pipeline chains
DELTA = 1                            # chain round stagger
NQ = 4                               # PSUM quarters per chunk (1024 wide)
QW = NA * NB // NQ                   # 1024
PS_BUFS, Q_BUFS, R_BUFS, QS_BUFS, C_BUFS = 2, 3, 10, 3, 8

FD = COLS // M_CHAINS                # columns per chain
# Per-quarter elementwise path assignment, one string of len NQ per chunk.
# A = ScalarE drain + DVE 2x mul, B = DVE direct from PSUM, C = GPSIMD direct.
PATHS = ["CBAB", "BCAB"] * 4


def _act_recip(nc, out, in_, bias=1.0):
    """out = 1 / (in_ + bias) on ScalarE (Reciprocal LUT, ~1.2e-5 rel)."""
    eng = nc.scalar
    ins = [eng.lower_ap(in_)]
    for arg in (bias, 1.0, 0.0):  # bias, scale, alpha
        ins.append(mybir.ImmediateValue(dtype=mybir.dt.float32, value=float(arg)))
    return eng.add_instruction(
        mybir.InstActivation(
            name=nc.get_next_instruction_name(),
            func=RECIP,
            ins=ins,
            outs=[eng.lower_ap(out)],
        )
    )


def _emit_core(ctx, tc, at16, btp, c_out):
    """Emit the per-core kernel body.

    at16: DRAM [2560, 64] fp16 = [wab-packed(512) | per-chain AT/BT rows].
    btp: DRAM [1024, 64] fp16 (BT again; loaded permuted batch-layout).
    c_out: [1024, 4096] fp16.
    """
    nc = tc.nc
    fd = COLS // M_CHAINS
    n_rounds = N_ROUNDS

    singles = ctx.enter_context(tc.tile_pool(name="singles", bufs=1))
    ps_pool = ctx.enter_context(tc.tile_pool(name="ps", bufs=PS_BUFS, space="PSUM"))
    q_pool = ctx.enter_context(tc.tile_pool(name="qps", bufs=Q_BUFS, space="PSUM"))
    r_pool = ctx.enter_context(tc.tile_pool(name="rp", bufs=R_BUFS))
    qs_pool = ctx.enter_context(tc.tile_pool(name="qsp", bufs=QS_BUFS))
    c_pool = ctx.enter_context(tc.tile_pool(name="cp", bufs=C_BUFS))

    # ---- static tiles -------------------------------------------------
    fd_ = COLS // M_CHAINS
    in1_t = singles.tile([P, 2 * P + 2 * fd_], F16, tag="in1")
    inN_t = [
        singles.tile([P, 2 * fd_], F16, name=f"inx{t}", tag=f"inx{t}")
        for t in range(1, M_CHAINS)
    ]
    att_c = [in1_t[:, 2 * P : 2 * P + fd_]] + [x[:, 0:fd_] for x in inN_t]
    btt_c = [in1_t[:, 2 * P + fd_ :]] + [x[:, fd_:] for x in inN_t]
    btp_t = singles.tile([P, COLS], F16, tag="btp_t")  # permuted batch layout
    wb_t = in1_t[:, 0:P]
    wa_t = in1_t[:, P : 2 * P]
    ra = singles.tile([NA, NA * NB], F16, tag="ra")   # expand: ra[i',(i,j)]
    afu = singles.tile([NA, B_CORE], F16, tag="afu")  # unpacked AF*^T
    bfs_c = [
        singles.tile([P, NB], F16, name=f"bfs{cc}", tag=f"bfs{cc}")
        for cc in range(N_CHUNK)
    ]

    af_c = [
        singles.tile([P, fd], F16, name=f"af{t}", tag=f"af{t}")
        for t in range(M_CHAINS)
    ]
    bf_c = [
        singles.tile([P, fd], F16, name=f"bf{t}", tag=f"bf{t}")
        for t in range(M_CHAINS)
    ]
    def bf_read(s, t):
        if s == 0:
            return btt_c[t]
        return bf_c[t]

    def bf_write(s, t):
        return bf_c[t]

    # ---- load inputs / constants --------------------------------------
    # at16 hosts [wabP | ATc0 | BTc0 | ATc1 | BTc1 | ...] rows, wab
    # pre-packed on the host in transpose layout, so ONE XBAR yields
    # wab+at+bt for chain 0; one more XBAR per later chain.
    nc.sync.dma_start_transpose(
        in1_t, at16[0 : COLS + 4 * fd].rearrange("(a b) j -> a (b j)", b=2)
    )
    for t in range(1, M_CHAINS):
        r0 = COLS + 4 * fd * t
        nc.sync.dma_start_transpose(
            inN_t[t - 1],
            at16[r0 : r0 + 4 * fd].rearrange("(a b) j -> a (b j)", b=2),
        )
    # permuted batch layout: btp_t[64g + b, cc*64 + j] = BT[cc*128 + 2b + g, j]
    btp4 = btp.rearrange("(cc b g) j -> g b cc j", g=2, b=NA)
    for g in range(2):
        nc.sync.dma_start(
            out=btp_t[g * NA : (g + 1) * NA, :].rearrange(
                "b (cc j) -> b cc j", j=NB
            ),
            in_=btp4[g],
        )

    # ---- build expand matrix on GPSIMD --------------------------------
    # ra[i', (i, j)] = K[i, j] if i == i' else 0
    nc.gpsimd.affine_select(
        out=ra.rearrange("p (i j) -> p i j", i=NA),
        in_=wa_t[0:NA, None, 0:NB].broadcast_to([NA, NA, NB]),
        compare_op=mybir.AluOpType.is_equal,
        fill=0.0,
        base=0,
        pattern=[[1, NA], [0, NB]],
        channel_multiplier=-1,
    )

    # ---- staggered schedule -------------------------------------------
    # Chain t runs its solve DELTA rounds behind chain t-1, so chain 0
    # finishes early and its chunks' C phase (the DMA stream) starts while
    # later chains still solve.  Per grid step: extraps, then all active
    # chains' A-steps, then B-steps, then one due C-chunk per chain --
    # keeping each in-order engine queue free of head-of-line blocking.
    cpch = fd // CPC              # chunks per chain

    def emit_A(s, t):
        ps1 = ps_pool.tile([P, fd], F32, tag="ps")
        nc.tensor.matmul(ps1, wb_t, bf_read(s, t), start=True, stop=True)
        r1 = r_pool.tile([P, fd], F16, tag="r")
        _act_recip(nc, r1, ps1, bias=1.0)
        nc.vector.tensor_mul(af_c[t], att_c[t], r1)
        if s == n_rounds - 1:
            # unpack this chain's AF*^T right away:
            # afu[j, cc*128 + 64g + b] = af[64g + j, cc*64 + b]
            cc0 = (t * fd) // CPC
            ncc = fd // CPC
            for g in range(2):
                nc.sync.dma_start(
                    out=afu.rearrange("j (cc g b) -> j cc g b", g=2, b=NA)[
                        :, cc0 : cc0 + ncc, g, :
                    ],
                    in_=af_c[t][g * NA : (g + 1) * NA, :].rearrange(
                        "j (cc b) -> j cc b", b=CPC
                    ),
                )

    def emit_B(s, t):
        ps2 = ps_pool.tile([P, fd], F32, tag="ps")
        nc.tensor.matmul(ps2, wa_t, af_c[t], start=True, stop=True)
        r2 = r_pool.tile([P, fd], F16, tag="r")
        _act_recip(nc, r2, ps2, bias=1.0)
        nc.vector.tensor_mul(bf_write(s, t), btt_c[t], r2)

    def emit_chunk(cc):
        au = afu[:, cc * P : (cc + 1) * P]
        # batch-layout final B-step: BFS[b, j] = BTP[b, j]/(1 + AF*@K)
        psb = ps_pool.tile([P, NB], F32, tag="ps")
        nc.tensor.matmul(psb, au, wa_t[0:NA, 0:NB], start=True, stop=True)
        rb = r_pool.tile([P, NB], F16, tag="r")
        _act_recip(nc, rb, psb, bias=1.0)
        nc.vector.tensor_mul(bfs_c[cc], btp_t[:, cc * NB : (cc + 1) * NB], rb)

        cs = c_pool.tile([P, NA * NB], F16, tag="c")
        for q in range(NQ):
            qp = q_pool.tile([P, QW], F32, tag="q")
            q0 = q * QW
            # matmul out must stay inside one 512-fp32 PSUM bank; the 64-col
            # starter also absorbs the PE low-p-state restart penalty
            for c0, c1 in ((0, 64), (64, 512), (512, QW)):
                nc.tensor.matmul(
                    qp[:, c0:c1], au, ra[:, q0 + c0 : q0 + c1],
                    start=True, stop=True,
                )
            ni = QW // NB  # i-values per quarter
            out_sl = cs[:, q0 : q0 + QW].rearrange("p (i j) -> p i j", i=ni)
            bcast = bfs_c[cc][:, None, :].broadcast_to([P, ni, NB])
            path = PATHS[cc][q]
            if path == "A":
                qs = qs_pool.tile([P, QW], F16, tag="qs")
                nc.scalar.copy(out=qs, in_=qp)
                nc.vector.tensor_mul(
                    out_sl, qs.rearrange("p (i j) -> p i j", i=ni), bcast
                )
            elif path == "B":
                nc.vector.tensor_mul(
                    out_sl, qp.rearrange("p (i j) -> p i j", i=ni), bcast
                )
            else:
                # GPSIMD cannot touch PSUM (BIR verifier); ScalarE drains
                # to SBUF first, GPSIMD multiplies from there.
                qs = qs_pool.tile([P, QW], F16, tag="qs")
                nc.scalar.copy(out=qs, in_=qp)
                nc.gpsimd.tensor_mul(
                    out_sl, qs.rearrange("p (i j) -> p i j", i=ni), bcast
                )
        # one DMA per quarter (earlier first transfer); the DRAM AP
        # un-permutes rows (p = 64g + b -> row 2b + g)
        for h in range(NQ):
            w = QW
            sl = slice(h * w, (h + 1) * w)
            nc.sync.dma_start(
                out=c_out[cc * P : (cc + 1) * P, sl].rearrange(
                    "(b g) x -> g b x", g=2
                ),
                in_=cs[:, sl],
            )

    n_gs = n_rounds + DELTA * (M_CHAINS - 1) + cpch
    for gs in range(n_gs):
        rounds_of = {t: gs - DELTA * t for t in range(M_CHAINS)}
        for t in range(M_CHAINS):
            if 0 <= rounds_of[t] < n_rounds:
                emit_A(rounds_of[t], t)
        for t in range(M_CHAINS):
            if 0 <= rounds_of[t] < n_rounds - 1:
                emit_B(rounds_of[t], t)
        for t in range(M_CHAINS):
            k = rounds_of[t] - n_rounds
            if 0 <= k < cpch:
                emit_chunk(t * cpch + k)


def build_nc(t_repeat=1, timing_mode=False):
    nc = bacc.Bacc("TRN2", target_bir_lowering=False, debug=False, num_devices=N_CORES)
    at16 = nc.dram_tensor(
        "at16", (2 * B_CORE + COLS, NA), F16, kind="ExternalInput"
    ).ap()
    btp = nc.dram_tensor("btp", (B_CORE, NB), F16, kind="ExternalInput").ap()

    with tile.TileContext(nc) as tc:
        if timing_mode:
            tok = nc.dram_tensor("tok", (1, NA), F16, kind="ExternalOutput").ap()
            with ExitStack() as octx:
                dram = octx.enter_context(
                    tc.tile_pool(name="cdram", bufs=1, space="DRAM")
                )
                c = dram.tile([B_CORE, NA * NB], F16, tag="cscratch")
                for _ in range(t_repeat):
                    with ExitStack() as ctx:
                        _emit_core(ctx, tc, at16, btp, c)
                nc.sync.dma_start(out=tok, in_=c[0:1, 0:NA])
        else:
            c = nc.dram_tensor(
                "c", (B_CORE, NA * NB), F16, kind="ExternalOutput"
            ).ap()
            for _ in range(t_repeat):
                with ExitStack() as ctx:
                    _emit_core(ctx, tc, at16, btp, c)
    nc.compile()
    return nc


_NC_CACHE = {}


def _get_nc(**kw):
    key = tuple(sorted(kw.items()))
    if key not in _NC_CACHE:
        _NC_CACHE[key] = build_nc(**kw)
    return _NC_CACHE[key]


def kernel(AT, BT, sqrt_K):
    AT16 = np.ascontiguousarray(AT, dtype=np.float16)
    BT16 = np.ascontiguousarray(BT, dtype=np.float16)
    K = np.ascontiguousarray(sqrt_K, dtype=np.float32) ** 2
    K16 = K.astype(np.float16)
    KT16 = np.ascontiguousarray(K16.T)
    wab = np.zeros((P, 2 * P), dtype=np.float16)
    wab[0:NB, 0:NA] = KT16          # wb block
    wab[NB:P, NA:P] = KT16
    wab[0:NA, P : P + NB] = K16     # wa block
    wab[NA:P, P + NB : 2 * P] = K16
    # pack for XBAR round trip: wab_packed[2c + g, j] = wab[64g + j, c]
    wab_packed = np.ascontiguousarray(
        wab.reshape(2, NA, 2 * P).transpose(2, 0, 1).reshape(COLS, NA)
    )

    nc = _get_nc()
    in_maps = [
        {
            "at16": np.concatenate(
                [wab_packed]
                + [
                    x
                    for t in range(M_CHAINS)
                    for x in (
                        AT16[
                            c * B_CORE + 2 * t * FD : c * B_CORE + 2 * (t + 1) * FD
                        ],
                        BT16[
                            c * B_CORE + 2 * t * FD : c * B_CORE + 2 * (t + 1) * FD
                        ],
                    )
                ]
            ),
            "btp": BT16[c * B_CORE : (c + 1) * B_CORE],
        }
        for c in range(N_CORES)
    ]
    res = run_bass_kernel_spmd(nc, in_maps, core_ids=list(range(N_CORES)))
    return np.concatenate(
        [
            r["c"].astype(np.float32).reshape(B_CORE, NA, NB)
            for r in res.results
        ],
        axis=0,
    )


# revision 56
# speedup vs baseline: 1.0787x; 1.0764x over previous
"""Trainium2 Bass kernel for nn_CompetitiveLayer_2 (competitive equilibrium layer).

Reference computation (per batch row b):
    K = sqrt_K ** 2                                  # (64, 64)
    repeat 30x:  AF = AT / (1 + BF @ K.T);  BF = BT / (1 + AF @ K)
    one more:    AF = AT / (1 + BF @ K.T);  BF = BT / (1 + AF @ K)
    C[b, i, j] = AF[b, i] * K[i, j] * BF[b, j]       # (B, 64, 64)

Sharding: pure data parallel over the batch dim, 1024 rows per core on 8 cores.

Per-core design (fp16 output; tolerance is 2e-2 scale-rel, this lands 8.1e-3;
sim makespan 53.2 us vs the 94.7 us fp32 baseline):
  - C is written to DRAM as fp16 (8 MB/core) -> DMA write floor ~23 us at
    the modeled 360 GB/s, half the fp32 floor.  The host upcasts to fp32.
  - State is transposed 2-group packed: X_T[64g + j, c] = X[2c + g, j].
    Inputs are uploaded as ONE fp16 tensor [wabP | ATc0 | BTc0 | ATc1 |
    BTc1] whose XBAR dma_start_transpose ([rows,64] viewed [rows/2,128])
    lands wab = [blkdiag(KT,KT) | blkdiag(K,K)] plus chain 0's at/bt in a
    single DMA (wab is host-packed to survive the XBAR round trip), so the
    solve starts ~3.6 us in; chain 1's xbar follows.  Early DMAs serialize
    (~2.3 us each through the DGE pipeline + completion-sem chaining), so
    instruction count on the startup critical path is minimized.
  - Solve: N_ROUNDS-1 plain fp16 fixed-point rounds + the final
    differentiable round (plain-6 error 7.8e-3 > Aitken gains here; the
    serial mm -> recip -> mul chain costs ~1.2 us/step regardless of
    width, so rounds, not engine busy, set the solve wall time).  Each
    step: PE matmul vs the blockdiag (fp16, 1 cyc/col), ScalarE
    Reciprocal LUT with bias=1 (PSUM fp32 -> SBUF fp16), DVE multiply in
    2x_1p fp16 mode.  M_CHAINS column chains pipeline the engines; chain
    t runs DELTA rounds behind chain t-1 so chain 0's C phase (and the
    output DMA stream) starts while chain 1 still solves.
  - Final round: the A-step produces AF*^T packed; two strided SBUF-SBUF
    DMAs (dispatched during the solve) unpack it to afu[64(i), 1024] with
    column order (chunk, g, b) <-> batch row 128*chunk + 2b + g.  Per
    chunk, a batch-layout matmul vs K + recip + mul produce BFS[b, j]
    fp16 (b in the same permuted order; the C output DMA un-permutes rows
    via its DRAM access pattern "(b g) x -> g b x").
  - C phase per chunk: Q = matmul(afu_chunk, ra) -> PSUM fp32 quarters
    (in 64/448/512-col pieces: outputs must stay in one 512-fp32 PSUM
    bank, and the tiny leading piece absorbs the PE's low-p-state restart
    penalty after an idle gap).  ra[i',(i,j)] = K[i,j] d_{ii'} fp16 is
    built once by GPSIMD affine_select during the solve.  Then
    cs = Q * BFS[b,j] (broadcast over i, j innermost) via one of three
    paths chosen per quarter to balance engines under the DMA roofline
    (GPSIMD cannot read PSUM, so its path needs the ScalarE drain too):
      A: ScalarE drains PSUM -> SBUF fp16, DVE multiplies in 2x_1p mode
      B: DVE multiplies straight from PSUM (1x)
      C: ScalarE drains, GPSIMD multiplies from SBUF
    One DMA per 1024-col quarter; the window runs at ~91% DMA duty.
"""

from contextlib import ExitStack

import numpy as np

import concourse.tile as tile
from concourse import bacc, mybir
from concourse.bass_utils import run_bass_kernel_spmd

F32 = mybir.dt.float32
F16 = mybir.dt.float16
RECIP = mybir.ActivationFunctionType.Reciprocal

P = 128          # SBUF partitions
NA = 64          # AF feature dim (i)
NB = 64          # BF feature dim (j)
B_TOTAL = 8192
N_CORES = 8
B_CORE = B_TOTAL // N_CORES          # 1024
N_CHUNK = B_CORE // P                # 8 chunks of 128 batch rows
COLS = B_CORE // 2                   # 512 packed columns (2 groups)
CPC = COLS // N_CHUNK                # 64 packed columns per chunk

N_ROUNDS = 5                         # fixed-point rounds (incl final)
OMEGA = 1.2                          # over-relaxation factor (SOR)
M_CHAINS = 2                         # solve pipeline chains
DELTA = 1                            # chain round stagger
NQ = 4                               # PSUM quarters per chunk (1024 wide)
QW = NA * NB // NQ                   # 1024
PS_BUFS, Q_BUFS, R_BUFS, QS_BUFS, C_BUFS = 2, 3, 10, 3, 8

FD = COLS // M_CHAINS                # columns per chain
# Per-quarter elementwise path assignment, one string of len NQ per chunk.
# A = ScalarE drain + DVE 2x mul, B = DVE direct from PSUM, C = GPSIMD direct.
PATHS = ["CBAB", "BCAB"] * 4


def _act_recip(nc, out, in_, bias=1.0, scale=1.0):
    """out = 1 / (in_*scale + bias) on ScalarE (Reciprocal LUT, ~1.2e-5 rel).

    With scale = bias = 1/omega this yields omega/(1 + in_): the SOR
    over-relaxation factor rides the activation immediates for free.
    """
    eng = nc.scalar
    ins = [eng.lower_ap(in_)]
    for arg in (bias, scale, 0.0):  # bias, scale, alpha
        ins.append(mybir.ImmediateValue(dtype=mybir.dt.float32, value=float(arg)))
    return eng.add_instruction(
        mybir.InstActivation(
            name=nc.get_next_instruction_name(),
            func=RECIP,
            ins=ins,
            outs=[eng.lower_ap(out)],
        )
    )


def _emit_core(ctx, tc, at16, btp, c_out):
    """Emit the per-core kernel body.

    at16: DRAM [2560, 64] fp16 = [wab-packed(512) | per-chain AT/BT rows].
    btp: DRAM [1024, 64] fp16 (BT again; loaded permuted batch-layout).
    c_out: [1024, 4096] fp16.
    """
    nc = tc.nc
    fd = COLS // M_CHAINS
    n_rounds = N_ROUNDS

    singles = ctx.enter_context(tc.tile_pool(name="singles", bufs=1))
    ps_pool = ctx.enter_context(tc.tile_pool(name="ps", bufs=PS_BUFS, space="PSUM"))
    q_pool = ctx.enter_context(tc.tile_pool(name="qps", bufs=Q_BUFS, space="PSUM"))
    r_pool = ctx.enter_context(tc.tile_pool(name="rp", bufs=R_BUFS))
    qs_pool = ctx.enter_context(tc.tile_pool(name="qsp", bufs=QS_BUFS))
    c_pool = ctx.enter_context(tc.tile_pool(name="cp", bufs=C_BUFS))

    # ---- static tiles -------------------------------------------------
    fd_ = COLS // M_CHAINS
    in1_t = singles.tile([P, 2 * P + 2 * fd_], F16, tag="in1")
    inN_t = [
        singles.tile([P, 2 * fd_], F16, name=f"inx{t}", tag=f"inx{t}")
        for t in range(1, M_CHAINS)
    ]
    att_c = [in1_t[:, 2 * P : 2 * P + fd_]] + [x[:, 0:fd_] for x in inN_t]
    btt_c = [in1_t[:, 2 * P + fd_ :]] + [x[:, fd_:] for x in inN_t]
    btp_t = singles.tile([P, COLS], F16, tag="btp_t")  # permuted batch layout
    wb_t = in1_t[:, 0:P]
    wa_t = in1_t[:, P : 2 * P]
    ra = singles.tile([NA, NA * NB], F16, tag="ra")   # expand: ra[i',(i,j)]
    afu = singles.tile([NA, B_CORE], F16, tag="afu")  # unpacked AF*^T
    bfs_c = [
        singles.tile([P, NB], F16, name=f"bfs{cc}", tag=f"bfs{cc}")
        for cc in range(N_CHUNK)
    ]

    af_c = [
        singles.tile([P, fd], F16, name=f"af{t}", tag=f"af{t}")
        for t in range(M_CHAINS)
    ]
    bf_c = [
        singles.tile([P, fd], F16, name=f"bf{t}", tag=f"bf{t}")
        for t in range(M_CHAINS)
    ]
    blA_c = [
        singles.tile([P, fd], F16, name=f"blA{t}", tag=f"blA{t}")
        for t in range(M_CHAINS)
    ]
    blB_c = [
        singles.tile([P, fd], F16, name=f"blB{t}", tag=f"blB{t}")
        for t in range(M_CHAINS)
    ]
    def bf_read(s, t):
        if s == 0:
            return btt_c[t]
        return bf_c[t]

    def bf_write(s, t):
        return bf_c[t]

    # ---- load inputs / constants --------------------------------------
    # at16 hosts [wabP | ATc0 | BTc0 | ATc1 | BTc1 | ...] rows, wab
    # pre-packed on the host in transpose layout, so ONE XBAR yields
    # wab+at+bt for chain 0; one more XBAR per later chain.
    nc.sync.dma_start_transpose(
        in1_t, at16[0 : COLS + 4 * fd].rearrange("(a b) j -> a (b j)", b=2)
    )
    for t in range(1, M_CHAINS):
        r0 = COLS + 4 * fd * t
        nc.sync.dma_start_transpose(
            inN_t[t - 1],
            at16[r0 : r0 + 4 * fd].rearrange("(a b) j -> a (b j)", b=2),
        )
    # permuted batch layout: btp_t[64g + b, cc*64 + j] = BT[cc*128 + 2b + g, j]
    btp4 = btp.rearrange("(cc b g) j -> g b cc j", g=2, b=NA)
    for g in range(2):
        nc.sync.dma_start(
            out=btp_t[g * NA : (g + 1) * NA, :].rearrange(
                "b (cc j) -> b cc j", j=NB
            ),
            in_=btp4[g],
        )

    # ---- build expand matrix on GPSIMD --------------------------------
    # ra[i', (i, j)] = K[i, j] if i == i' else 0
    nc.gpsimd.affine_select(
        out=ra.rearrange("p (i j) -> p i j", i=NA),
        in_=wa_t[0:NA, None, 0:NB].broadcast_to([NA, NA, NB]),
        compare_op=mybir.AluOpType.is_equal,
        fill=0.0,
        base=0,
        pattern=[[1, NA], [0, NB]],
        channel_multiplier=-1,
    )

    # ---- staggered schedule -------------------------------------------
    # Chain t runs its solve DELTA rounds behind chain t-1, so chain 0
    # finishes early and its chunks' C phase (the DMA stream) starts while
    # later chains still solve.  Per grid step: extraps, then all active
    # chains' A-steps, then B-steps, then one due C-chunk per chain --
    # keeping each in-order engine queue free of head-of-line blocking.
    cpch = fd // CPC              # chunks per chain

    def emit_A(s, t):
        # SOR: af <- omega*at*(1/(1+ps)) + (1-omega)*af_old on rounds
        # 1..n-2; the (1-omega) term is computed off the critical path and
        # omega rides the activation scale/bias immediates.
        blend = 1 <= s <= n_rounds - 2
        if blend:
            nc.vector.tensor_scalar_mul(
                out=blA_c[t], in0=af_c[t], scalar1=1.0 - OMEGA
            )
        ps1 = ps_pool.tile([P, fd], F32, tag="ps")
        nc.tensor.matmul(ps1, wb_t, bf_read(s, t), start=True, stop=True)
        r1 = r_pool.tile([P, fd], F16, tag="r")
        if blend:
            _act_recip(nc, r1, ps1, bias=1.0 / OMEGA, scale=1.0 / OMEGA)
        else:
            _act_recip(nc, r1, ps1, bias=1.0)
        nc.vector.tensor_mul(af_c[t], att_c[t], r1)
        if blend:
            nc.vector.tensor_add(out=af_c[t], in0=af_c[t], in1=blA_c[t])
        if s == n_rounds - 1:
            # unpack this chain's AF*^T right away:
            # afu[j, cc*128 + 64g + b] = af[64g + j, cc*64 + b]
            cc0 = (t * fd) // CPC
            ncc = fd // CPC
            for g in range(2):
                nc.sync.dma_start(
                    out=afu.rearrange("j (cc g b) -> j cc g b", g=2, b=NA)[
                        :, cc0 : cc0 + ncc, g, :
                    ],
                    in_=af_c[t][g * NA : (g + 1) * NA, :].rearrange(
                        "j (cc b) -> j cc b", b=CPC
                    ),
                )

    def emit_B(s, t):
        ps2 = ps_pool.tile([P, fd], F32, tag="ps")
        nc.tensor.matmul(ps2, wa_t, af_c[t], start=True, stop=True)
        r2 = r_pool.tile([P, fd], F16, tag="r")
        _act_recip(nc, r2, ps2, bias=1.0)
        nc.vector.tensor_mul(bf_write(s, t), btt_c[t], r2)

    def emit_chunk(cc):
        au = afu[:, cc * P : (cc + 1) * P]
        # batch-layout final B-step: BFS[b, j] = BTP[b, j]/(1 + AF*@K)
        psb = ps_pool.tile([P, NB], F32, tag="ps")
        nc.tensor.matmul(psb, au, wa_t[0:NA, 0:NB], start=True, stop=True)
        rb = r_pool.tile([P, NB], F16, tag="r")
        _act_recip(nc, rb, psb, bias=1.0)
        nc.vector.tensor_mul(bfs_c[cc], btp_t[:, cc * NB : (cc + 1) * NB], rb)

        cs = c_pool.tile([P, NA * NB], F16, tag="c")
        for q in range(NQ):
            qp = q_pool.tile([P, QW], F32, tag="q")
            q0 = q * QW
            # matmul out must stay inside one 512-fp32 PSUM bank; the 64-col
            # starter also absorbs the PE low-p-state restart penalty
            for c0, c1 in ((0, 64), (64, 512), (512, QW)):
                nc.tensor.matmul(
                    qp[:, c0:c1], au, ra[:, q0 + c0 : q0 + c1],
                    start=True, stop=True,
                )
            ni = QW // NB  # i-values per quarter
            out_sl = cs[:, q0 : q0 + QW].rearrange("p (i j) -> p i j", i=ni)
            bcast = bfs_c[cc][:, None, :].broadcast_to([P, ni, NB])
            path = PATHS[cc][q]
            if path == "A":
                qs = qs_pool.tile([P, QW], F16, tag="qs")
                nc.scalar.copy(out=qs, in_=qp)
                nc.vector.tensor_mul(
                    out_sl, qs.rearrange("p (i j) -> p i j", i=ni), bcast
                )
            elif path == "B":
                nc.vector.tensor_mul(
                    out_sl, qp.rearrange("p (i j) -> p i j", i=ni), bcast
                )
            else:
                # GPSIMD cannot touch PSUM (BIR verifier); ScalarE drains
                # to SBUF first, GPSIMD multiplies from there.
                qs = qs_pool.tile([P, QW], F16, tag="qs")
                nc.scalar.copy(out=qs, in_=qp)
                nc.gpsimd.tensor_mul(
                    out_sl, qs.rearrange("p (i j) -> p i j", i=ni), bcast
                )
        # one DMA per quarter (earlier first transfer); the DRAM AP
        # un-permutes rows (p = 64g + b -> row 2b + g)
        for h in range(NQ):
            w = QW
            sl = slice(h * w, (h + 1) * w)
            nc.sync.dma_start(
                out=c_out[cc * P : (cc + 1) * P, sl].rearrange(
                    "(b g) x -> g b x", g=2
                ),
                in_=cs[:, sl],
            )

    n_gs = n_rounds + DELTA * (M_CHAINS - 1) + cpch
    for gs in range(n_gs):
        rounds_of = {t: gs - DELTA * t for t in range(M_CHAINS)}
        for t in range(M_CHAINS):
            if 0 <= rounds_of[t] < n_rounds:
                emit_A(rounds_of[t], t)
        for t in range(M_CHAINS):
            if 0 <= rounds_of[t] < n_rounds - 1:
                emit_B(rounds_of[t], t)
        for t in range(M_CHAINS):
            k = rounds_of[t] - n_rounds
            if 0 <= k < cpch:
                emit_chunk(t * cpch + k)


def build_nc(t_repeat=1, timing_mode=False):
    nc = bacc.Bacc("TRN2", target_bir_lowering=False, debug=False, num_devices=N_CORES)
    at16 = nc.dram_tensor(
        "at16", (2 * B_CORE + COLS, NA), F16, kind="ExternalInput"
    ).ap()
    btp = nc.dram_tensor("btp", (B_CORE, NB), F16, kind="ExternalInput").ap()

    with tile.TileContext(nc) as tc:
        if timing_mode:
            tok = nc.dram_tensor("tok", (1, NA), F16, kind="ExternalOutput").ap()
            with ExitStack() as octx:
                dram = octx.enter_context(
                    tc.tile_pool(name="cdram", bufs=1, space="DRAM")
                )
                c = dram.tile([B_CORE, NA * NB], F16, tag="cscratch")
                for _ in range(t_repeat):
                    with ExitStack() as ctx:
                        _emit_core(ctx, tc, at16, btp, c)
                nc.sync.dma_start(out=tok, in_=c[0:1, 0:NA])
        else:
            c = nc.dram_tensor(
                "c", (B_CORE, NA * NB), F16, kind="ExternalOutput"
            ).ap()
            for _ in range(t_repeat):
                with ExitStack() as ctx:
                    _emit_core(ctx, tc, at16, btp, c)
    nc.compile()
    return nc


_NC_CACHE = {}


def _get_nc(**kw):
    key = tuple(sorted(kw.items()))
    if key not in _NC_CACHE:
        _NC_CACHE[key] = build_nc(**kw)
    return _NC_CACHE[key]


def kernel(AT, BT, sqrt_K):
    AT16 = np.ascontiguousarray(AT, dtype=np.float16)
    BT16 = np.ascontiguousarray(BT, dtype=np.float16)
    K = np.ascontiguousarray(sqrt_K, dtype=np.float32) ** 2
    K16 = K.astype(np.float16)
    KT16 = np.ascontiguousarray(K16.T)
    wab = np.zeros((P, 2 * P), dtype=np.float16)
    wab[0:NB, 0:NA] = KT16          # wb block
    wab[NB:P, NA:P] = KT16
    wab[0:NA, P : P + NB] = K16     # wa block
    wab[NA:P, P + NB : 2 * P] = K16
    # pack for XBAR round trip: wab_packed[2c + g, j] = wab[64g + j, c]
    wab_packed = np.ascontiguousarray(
        wab.reshape(2, NA, 2 * P).transpose(2, 0, 1).reshape(COLS, NA)
    )

    nc = _get_nc()
    in_maps = [
        {
            "at16": np.concatenate(
                [wab_packed]
                + [
                    x
                    for t in range(M_CHAINS)
                    for x in (
                        AT16[
                            c * B_CORE + 2 * t * FD : c * B_CORE + 2 * (t + 1) * FD
                        ],
                        BT16[
                            c * B_CORE + 2 * t * FD : c * B_CORE + 2 * (t + 1) * FD
                        ],
                    )
                ]
            ),
            "btp": BT16[c * B_CORE : (c + 1) * B_CORE],
        }
        for c in range(N_CORES)
    ]
    res = run_bass_kernel_spmd(nc, in_maps, core_ids=list(range(N_CORES)))
    return np.concatenate(
        [
            r["c"].astype(np.float32).reshape(B_CORE, NA, NB)
            for r in res.results
        ],
        axis=0,
    )
